# revision 39
# baseline (speedup 1.0000x reference)
"""DepthDC3x3 dynamic depthwise conv — Trainium2 Bass kernel, 8 NeuronCores.

Per-sample pipeline (data-parallel over batch N=8 -> one sample per core):
  A  = conv3x3(y, w_gk1) + b_gk1                    # [64,128,128]
  K  = conv1x1(A, w_gk2) + b_gk2                    # [576,128,128] -> per-tap
  r  = sum_k K[:,k] * shift_k(x)                    # dynamic depthwise 3x3
  out= conv3x3(r, w_fuse) + b_fuse                  # [64,128,128]

Key design points (v6):
  * Half-split layout: SBUF partitions = (channel 0..63) x (image half).
  * Inputs are pre-padded + pre-cast to bf16 on the HOST into the exact
    on-chip layout ([128 part, 66, 130], plus a halves-swapped copy of x),
    so the device does zero casts/swaps: 12 chunked HWDGE loads (y on the
    sync ring, x + xswp on the scalar ring) feed compute directly.
  * conv1 window-pairs produce one normal + one halves-swapped ("inverted")
    PSUM tile; inversion is never fixed up downstream.
  * Stage C: conv2 PSUM tiles are [128,1024] 2-bank (windows w, w+2, same
    parity) -> ONE ACT eviction and ONE DVE multiply per tap. Separate
    accumulation chains per parity; each chain's last add writes straight
    into the padded accumulator (rpn / rpi) - no fold, no swap DMA.
  * conv3 exploits linearity: 9 taps over rpn + 9 over rpi (own PSUM tile,
    quadrant-swapped outputs), combined at eviction (ACT evict + DVE add).
    conv3 pairs 1-4 run interleaved with late stage_c's; only pair 0 needs
    the last halo, so the tail is short.
  * bf16 matmuls, 2x2 tile_position quadrants -> 4x PE concurrency.
"""

import threading

import numpy as np
import ml_dtypes

import concourse.bacc as bacc
import concourse.mybir as mybir
from concourse.tile import TileContext

BF = mybir.dt.bfloat16
F32 = mybir.dt.float32

C = 64            # channels
HR = 64           # rows per half
W = 128           # image width
PW = W + 2        # padded width
PR = HR + 2       # padded rows per half
NWIN = 16         # windows per half (4 rows x 128 cols each)
WPX = 512
RW = 4            # rows per window
KS = 3

# row chunks of the padded-row index space (0..66)
CHUNKS = ((0, 18), (18, 34), (34, 50), (50, 66))

_lock = threading.Lock()
_cache = {}


def _tap_off(t):
    return divmod(t, KS)


def build_nc():
    nc = bacc.Bacc("TRN2", target_bir_lowering=False, debug=False)

    xp_d = nc.dram_tensor("xpd", [128, PR, PW], BF, kind="ExternalInput")
    xs_d = nc.dram_tensor("xsd", [128, PR, PW], BF, kind="ExternalInput")
    yp_d = nc.dram_tensor("ypd", [128, PR, PW], BF, kind="ExternalInput")
    w1_d = nc.dram_tensor("w1d", [128, 9 * C], BF, kind="ExternalInput")
    w2_d = nc.dram_tensor("w2d", [128, 9 * C], BF, kind="ExternalInput")
    w3_d = nc.dram_tensor("w3d", [128, 9 * C], BF, kind="ExternalInput")
    b1_d = nc.dram_tensor("b1d", [128, 1], F32, kind="ExternalInput")
    b2_d = nc.dram_tensor("b2d", [128, 9], F32, kind="ExternalInput")
    b3_d = nc.dram_tensor("b3d", [128, 1], F32, kind="ExternalInput")
    out_d = nc.dram_tensor("out", [C, 2 * HR, W], F32, kind="ExternalOutput")

    with TileContext(nc) as tc:
        with (
            tc.tile_pool(name="big", bufs=1) as big,
            tc.tile_pool(name="wpool", bufs=1) as wpool,
            tc.tile_pool(name="kev", bufs=6) as kevp,
            tc.tile_pool(name="prod", bufs=6) as prodp,
            tc.tile_pool(name="accs", bufs=4) as accp,
            tc.tile_pool(name="stag", bufs=3) as stagp,
            tc.tile_pool(name="ps13", bufs=2, space="PSUM") as ps13p,
            tc.tile_pool(name="psC", bufs=2, space="PSUM") as psCp,
        ):
            ypad = big.tile([128, PR, PW], BF)
            xpad = big.tile([128, PR, PW], BF)
            xswp = big.tile([128, PR, PW], BF)
            a2 = big.tile([128, NWIN * WPX], BF)
            rpn = big.tile([128, PR, PW], BF)
            rpi = big.tile([128, PR, PW], BF)

            w1t = wpool.tile([128, 9 * C], BF)
            w2t = wpool.tile([128, 9 * C], BF)
            w3t = wpool.tile([128, 9 * C], BF)
            b1t = wpool.tile([128, 1], F32)
            b2t = wpool.tile([128, 9], F32)
            b3t = wpool.tile([128, 1], F32)

            def ldchunk(dst, src, k, q):
                r0, r1 = CHUNKS[k]
                q.dma_start(out=dst[:, r0:r1, :], in_=src[:, r0:r1, :])

            ident = mybir.ActivationFunctionType.Identity

            def conv9(src, wt, psLR, w0):
                """3x3 conv quadrants for adjacent window-pair (w0, w0+1).
                psLR [128,1024]: [:,0:512] win w0 normal; [:,512:1024] win
                w0+1 inverted."""
                r0, r1 = RW * w0, RW * (w0 + 1)
                for t in range(9):
                    dy, dx = _tap_off(t)
                    st, sp = (t == 0), (t == 8)
                    lhsL = wt[0:64, t * C:(t + 1) * C]
                    lhsH = wt[64:128, t * C:(t + 1) * C]
                    nc.tensor.matmul(psLR[0:64, 0:512], lhsL,
                                     src[0:64, r0 + dy:r0 + dy + 4, dx:dx + 128],
                                     start=st, stop=sp, tile_position=(0, 0),
                                     skip_group_check=True)
                    nc.tensor.matmul(psLR[64:128, 0:512], lhsH,
                                     src[64:128, r0 + dy:r0 + dy + 4, dx:dx + 128],
                                     start=st, stop=sp, tile_position=(64, 64),
                                     skip_group_check=True)
                    nc.tensor.matmul(psLR[64:128, 512:1024], lhsL,
                                     src[0:64, r1 + dy:r1 + dy + 4, dx:dx + 128],
                                     start=st, stop=sp, tile_position=(0, 64),
                                     skip_group_check=True)
                    nc.tensor.matmul(psLR[0:64, 512:1024], lhsH,
                                     src[64:128, r1 + dy:r1 + dy + 4, dx:dx + 128],
                                     start=st, stop=sp, tile_position=(64, 0),
                                     skip_group_check=True)

            def conv9_swapped(src, wt, psLR, w0):
                """conv9 but every output lands in the opposite partition
                half — consumes the inverted accumulator without a swap."""
                r0, r1 = RW * w0, RW * (w0 + 1)
                for t in range(9):
                    dy, dx = _tap_off(t)
                    st, sp = (t == 0), (t == 8)
                    lhsL = wt[0:64, t * C:(t + 1) * C]
                    lhsH = wt[64:128, t * C:(t + 1) * C]
                    nc.tensor.matmul(psLR[64:128, 0:512], lhsL,
                                     src[0:64, r0 + dy:r0 + dy + 4, dx:dx + 128],
                                     start=st, stop=sp, tile_position=(0, 64),
                                     skip_group_check=True)
                    nc.tensor.matmul(psLR[0:64, 0:512], lhsH,
                                     src[64:128, r0 + dy:r0 + dy + 4, dx:dx + 128],
                                     start=st, stop=sp, tile_position=(64, 0),
                                     skip_group_check=True)
                    nc.tensor.matmul(psLR[0:64, 512:1024], lhsL,
                                     src[0:64, r1 + dy:r1 + dy + 4, dx:dx + 128],
                                     start=st, stop=sp, tile_position=(0, 0),
                                     skip_group_check=True)
                    nc.tensor.matmul(psLR[64:128, 512:1024], lhsH,
                                     src[64:128, r1 + dy:r1 + dy + 4, dx:dx + 128],
                                     start=st, stop=sp, tile_position=(64, 64),
                                     skip_group_check=True)

            def conv1_pair(p):
                w0 = 2 * p
                psLR = ps13p.tile([128, 2 * WPX], F32, tag="ps13",
                                  name=f"c1ps{p}")
                conv9(ypad, w1t, psLR, w0)
                nc.scalar.activation(a2[:, w0 * WPX:(w0 + 2) * WPX],
                                     psLR[:, :], ident, bias=b1t[:, 0:1])

            def x2win(xsrc, wA, dy, dx):
                """[128, 2, 4, 128] view covering same-parity windows wA,
                wA+2 at tap offset (dy, dx)."""
                r0 = RW * wA + dy
                return xsrc[:, r0:r0 + 12, dx:dx + 128].rearrange(
                    "p (b r) c -> p b r c", b=3)[:, 0::2]

            def stage_c(wA, cbs=None):
                """conv2 + dynamic multiply-sum for windows (wA, wA+2).
                Parity-split chains; last add of each chain lands in
                rpn/rpi. cbs = {jp: callback} sprinkles conv3 emission
                between jp groups so its PE blocks hide under ACT's lag."""
                wB = wA + 2
                invA = (wA % 2 == 1)
                size = {invA: 5, (not invA): 4}
                cnt = {False: 0, True: 0}
                acc = {False: None, True: None}
                r0 = RW * wA
                rout = {
                    False: rpn[:, r0 + 1:r0 + 13, 1:129].rearrange(
                        "p (b r) c -> p b r c", b=3)[:, 0::2],
                    True: rpi[:, r0 + 1:r0 + 13, 1:129].rearrange(
                        "p (b r) c -> p b r c", b=3)[:, 0::2],
                }

                for jp in range(5):
                    ta = 2 * jp
                    taps = (ta,) if ta == 8 else (ta, ta + 1)
                    psA = psCp.tile([128, 2 * WPX], F32, tag="ps2",
                                    name=f"psA{wA}_{jp}")
                    lhsA_l = w2t[0:64, ta * C:(ta + 1) * C]
                    lhsA_h = w2t[64:128, ta * C:(ta + 1) * C]
                    psB = None
                    if len(taps) == 2:
                        tb = ta + 1
                        psB = psCp.tile([128, 2 * WPX], F32, tag="ps2",
                                        name=f"psB{wA}_{jp}")
                        lhsB_l = w2t[0:64, tb * C:(tb + 1) * C]
                        lhsB_h = w2t[64:128, tb * C:(tb + 1) * C]
                    for wi, wv in ((0, wA), (1, wB)):
                        awin = a2[:, wv * WPX:(wv + 1) * WPX]
                        sl = slice(wi * WPX, (wi + 1) * WPX)
                        nc.tensor.matmul(psA[0:64, sl], lhsA_l, awin[0:64, :],
                                         tile_position=(0, 0),
                                         skip_group_check=True)
                        nc.tensor.matmul(psA[64:128, sl], lhsA_h,
                                         awin[64:128, :],
                                         tile_position=(64, 64),
                                         skip_group_check=True)
                        if psB is not None:
                            nc.tensor.matmul(psB[64:128, sl], lhsB_l,
                                             awin[0:64, :],
                                             tile_position=(0, 64),
                                             skip_group_check=True)
                            nc.tensor.matmul(psB[0:64, sl], lhsB_h,
                                             awin[64:128, :],
                                             tile_position=(64, 0),
                                             skip_group_check=True)

                    for t in taps:
                        ps = psA if t == ta else psB
                        inv = invA if t == ta else (not invA)
                        dy, dx = _tap_off(t)
                        xop = x2win(xswp if inv else xpad, wA, dy, dx)
                        cnt[inv] += 1
                        first = (cnt[inv] == 1)
                        last = (cnt[inv] == size[inv])
                        if first:
                            acc[inv] = accp.tile([128, 2 * WPX], BF, tag="acc",
                                                 name=f"ac{wA}_{int(inv)}")
                            tgt = acc[inv]
                        else:
                            tgt = prodp.tile([128, 2 * WPX], BF, tag="prod",
                                             name=f"pr{wA}_{t}")
                        tgt4 = tgt[:, :].rearrange("p (b r f) -> p b r f",
                                                   b=2, r=4)
                        kev = kevp.tile([128, 2 * WPX], BF, tag="kev",
                                        name=f"kv{wA}_{t}")
                        nc.scalar.activation(kev[:, :], ps[:, :], ident,
                                             bias=b2t[:, t:t + 1])
                        nc.vector.tensor_mul(
                            out=tgt4,
                            in0=kev[:, :].rearrange(
                                "p (b r f) -> p b r f", b=2, r=4),
                            in1=xop)
                        if not first:
                            if last:
                                nc.vector.tensor_add(
                                    out=rout[inv],
                                    in0=acc[inv][:, :].rearrange(
                                        "p (b r f) -> p b r f", b=2, r=4),
                                    in1=tgt4)
                            else:
                                nc.vector.tensor_add(out=acc[inv][:, :],
                                                     in0=acc[inv][:, :],
                                                     in1=tgt[:, :])
                    if cbs and jp in cbs:
                        cbs[jp]()

            def halosA():
                # available after stage_c(0): rpn win0 h1 row0, rpi h1 row0
                nc.sync.dma_start(out=rpn[0:64, 65:66, :],
                                  in_=rpn[64:128, 1:2, :])
                nc.sync.dma_start(out=rpi[64:128, 65:66, :],
                                  in_=rpi[0:64, 1:2, :])

            def halosB():
                # available after stage_c(13): win15 h0 row63
                nc.sync.dma_start(out=rpn[64:128, 0:1, :],
                                  in_=rpn[0:64, 64:65, :])
                nc.sync.dma_start(out=rpi[0:64, 0:1, :],
                                  in_=rpi[64:128, 64:65, :])

            # conv3 over rpn + rpi for windows (2p, 2p+1); the two source
            # streams accumulate in separate PSUM tiles, combined at
            # eviction: st = (psN + b3) [ACT], st += psI [DVE]. Split into
            # three emission pieces so the MM blocks can hide inside
            # stage_c's jp cadence.
            c3ps = {}

            def c3n(p):
                psN = ps13p.tile([128, 2 * WPX], F32, tag="ps13",
                                 name=f"c3n{p}")
                conv9(rpn, w3t, psN, 2 * p)
                c3ps[p] = psN

            def c3i(p):
                psI = ps13p.tile([128, 2 * WPX], F32, tag="ps13",
                                 name=f"c3i{p}")
                conv9_swapped(rpi, w3t, psI, 2 * p)
                c3ps[p] = (c3ps[p], psI)

            def c3fin(p):
                psN, psI = c3ps.pop(p)
                w0 = 2 * p
                st_t = stagp.tile([128, 2 * WPX], F32, tag="stag",
                                  name=f"st{p}")
                nc.scalar.activation(st_t[:, :], psN[:, :], ident,
                                     bias=b3t[:, 0:1])
                nc.vector.tensor_add(out=st_t[:, :], in0=st_t[:, :],
                                     in1=psI[:, :])
                ra, rb = RW * w0, RW * (w0 + 1)
                nc.sync.dma_start(out=out_d[:, ra:ra + 4, :],
                                  in_=st_t[0:64, 0:512])
                nc.sync.dma_start(out=out_d[:, HR + ra:HR + ra + 4, :],
                                  in_=st_t[64:128, 0:512])
                nc.sync.dma_start(out=out_d[:, HR + rb:HR + rb + 4, :],
                                  in_=st_t[0:64, 512:1024])
                nc.sync.dma_start(out=out_d[:, rb:rb + 4, :],
                                  in_=st_t[64:128, 512:1024])

            def conv3_pair(p):
                c3n(p)
                c3i(p)
                c3fin(p)

            # ---- emission schedule ----
            ldchunk(ypad, yp_d, 0, nc.sync)
            for td, tt in ((w1_d, w1t), (w2_d, w2t), (w3_d, w3t),
                           (b1_d, b1t), (b2_d, b2t), (b3_d, b3t)):
                nc.sync.dma_start(out=tt[:], in_=td[:])

            nc.vector.memset(rpn[0:64, 0:1, :], 0.0)
            nc.vector.memset(rpn[64:128, 65:66, :], 0.0)
            nc.vector.memset(rpn[:, :, 0:1], 0.0)
            nc.vector.memset(rpn[:, :, 129:130], 0.0)
            nc.vector.memset(rpi[0:64, 65:66, :], 0.0)
            nc.vector.memset(rpi[64:128, 0:1, :], 0.0)
            nc.vector.memset(rpi[:, :, 0:1], 0.0)
            nc.vector.memset(rpi[:, :, 129:130], 0.0)

            for k in range(4):
                ldchunk(xpad, xp_d, k, nc.scalar)
                ldchunk(xswp, xs_d, k, nc.scalar)
                if k > 0:
                    ldchunk(ypad, yp_d, k, nc.sync)

            conv1_pair(0)
            conv1_pair(1)
            conv1_pair(2)
            conv1_pair(3)
            stage_c(0)
            halosA()
            conv1_pair(4)
            stage_c(1)
            conv1_pair(5)
            conv1_pair(6)
            stage_c(4)
            conv1_pair(7)
            stage_c(5, {3: lambda: c3n(1)})
            stage_c(8, {1: lambda: c3i(1), 2: lambda: c3fin(1),
                        3: lambda: c3n(2)})
            stage_c(9, {1: lambda: c3i(2), 2: lambda: c3fin(2),
                        3: lambda: c3n(3)})
            stage_c(13, {1: lambda: c3i(3), 2: lambda: c3fin(3),
                         3: lambda: c3n(4)})
            halosB()
            stage_c(12, {1: lambda: c3i(4), 2: lambda: c3fin(4),
                         3: lambda: c3n(0)})
            c3i(0)
            c3fin(0)
            conv3_pair(5)
            conv3_pair(6)
            conv3_pair(7)

    nc.compile()
    return nc


def _prep_weights(w_gk1, b_gk1, w_gk2, b_gk2, w_fuse, b_fuse):
    bf = ml_dtypes.bfloat16

    def conv_lhst(wc):
        l = np.empty((128, 9 * C), dtype=bf)
        for t in range(9):
            dy, dx = _tap_off(t)
            m = wc[:, :, dy, dx].T.astype(bf)  # [I, O] lhsT
            l[0:64, t * C:(t + 1) * C] = m
            l[64:128, t * C:(t + 1) * C] = m
        return l

    w1d = conv_lhst(np.asarray(w_gk1))
    w3d = conv_lhst(np.asarray(w_fuse))

    w2 = np.asarray(w_gk2).reshape(C * 9, C)
    w2d = np.empty((128, 9 * C), dtype=bf)
    for t in range(9):
        m = w2[t::9, :].T.astype(bf)
        w2d[0:64, t * C:(t + 1) * C] = m
        w2d[64:128, t * C:(t + 1) * C] = m

    b1 = np.asarray(b_gk1, np.float32)
    b3 = np.asarray(b_fuse, np.float32)
    b1d = np.concatenate([b1, b1]).reshape(128, 1)
    b3d = np.concatenate([b3, b3]).reshape(128, 1)
    b2 = np.asarray(b_gk2, np.float32).reshape(C, 9)
    b2d = np.concatenate([b2, b2], axis=0)
    return w1d, w2d, w3d, b1d, b2d, b3d


def _prep_pad(img):
    """[64,128,128] fp32 -> ([128,66,130] bf16 padded, halves-swapped)."""
    bf = ml_dtypes.bfloat16
    b = np.asarray(img, np.float32).astype(bf)
    pad = np.zeros((128, PR, PW), dtype=bf)
    pad[0:64, 1:66, 1:129] = b[:, 0:65]       # h0: img rows 0..64
    pad[64:128, 0:65, 1:129] = b[:, 63:128]   # h1: img rows 63..127
    swp = np.ascontiguousarray(
        np.concatenate([pad[64:128], pad[0:64]], axis=0))
    return pad, swp


def kernel(x, y, w_gk1, b_gk1, w_gk2, b_gk2, w_fuse, b_fuse):
    from concourse.bass_utils import run_bass_kernel_spmd

    with _lock:
        if "nc" not in _cache:
            _cache["nc"] = build_nc()
    nc = _cache["nc"]

    w1d, w2d, w3d, b1d, b2d, b3d = _prep_weights(
        w_gk1, b_gk1, w_gk2, b_gk2, w_fuse, b_fuse)

    x = np.asarray(x, np.float32)
    y = np.asarray(y, np.float32)
    n = x.shape[0]
    assert n == 8, f"expected batch 8, got {n}"
    in_maps = []
    for i in range(n):
        xpd, xsd = _prep_pad(x[i])
        ypd, _ = _prep_pad(y[i])
        in_maps.append({
            "xpd": xpd, "xsd": xsd, "ypd": ypd,
            "w1d": w1d, "w2d": w2d, "w3d": w3d,
            "b1d": b1d, "b2d": b2d, "b3d": b3d,
        })
    res = run_bass_kernel_spmd(nc, in_maps, core_ids=list(range(n)))
    return np.stack([res.results[i]["out"] for i in range(n)], axis=0)


# revision 42
# speedup vs baseline: 1.1836x; 1.1836x over previous
"""DepthDC3x3 dynamic depthwise conv — Trainium2 Bass kernel, 8 NeuronCores.

Per-sample pipeline (data-parallel over batch N=8 -> one sample per core):
  A  = conv3x3(y, w_gk1) + b_gk1                    # [64,128,128]
  K  = conv1x1(A, w_gk2) + b_gk2                    # [576,128,128] -> per-tap
  r  = sum_k K[:,k] * shift_k(x)                    # dynamic depthwise 3x3
  out= conv3x3(r, w_fuse) + b_fuse                  # [64,128,128]

Key design points (v6):
  * Half-split layout: SBUF partitions = (channel 0..63) x (image half).
  * Inputs are pre-padded + pre-cast to bf16 on the HOST into the exact
    on-chip layout ([128 part, 66, 130], plus a halves-swapped copy of x),
    so the device does zero casts/swaps: 12 chunked HWDGE loads (y on the
    sync ring, x + xswp on the scalar ring) feed compute directly.
  * conv1 window-pairs produce one normal + one halves-swapped ("inverted")
    PSUM tile; inversion is never fixed up downstream.
  * Stage C: conv2 PSUM tiles are [128,1024] 2-bank (windows w, w+2, same
    parity) -> ONE ACT eviction and ONE DVE multiply per tap. Separate
    accumulation chains per parity; each chain's last add writes straight
    into the padded accumulator (rpn / rpi) - no fold, no swap DMA.
  * conv3 exploits linearity: 9 taps over rpn + 9 over rpi (own PSUM tile,
    quadrant-swapped outputs), combined at eviction (ACT evict + DVE add).
    conv3 pairs 1-4 run interleaved with late stage_c's; only pair 0 needs
    the last halo, so the tail is short.
  * bf16 matmuls, 2x2 tile_position quadrants -> 4x PE concurrency.
"""

import threading

import numpy as np
import ml_dtypes

import concourse.bacc as bacc
import concourse.mybir as mybir
from concourse.tile import TileContext

BF = mybir.dt.bfloat16
F32 = mybir.dt.float32

C = 64            # channels
HR = 64           # rows per half
W = 128           # image width
PW = W + 2        # padded width
PR = HR + 2       # padded rows per half
NWIN = 16         # windows per half (4 rows x 128 cols each)
WPX = 512
RW = 4            # rows per window
KS = 3

# row chunks of the padded-row index space (0..66)
CHUNKS = ((0, 18), (18, 34), (34, 50), (50, 66))

_lock = threading.Lock()
_cache = {}


def _tap_off(t):
    return divmod(t, KS)


def build_nc():
    nc = bacc.Bacc("TRN2", target_bir_lowering=False, debug=False)

    xp_d = nc.dram_tensor("xpd", [128, PR, PW], BF, kind="ExternalInput")
    xs_d = nc.dram_tensor("xsd", [128, PR, PW], BF, kind="ExternalInput")
    yp_d = nc.dram_tensor("ypd", [128, PR, PW], BF, kind="ExternalInput")
    w1_d = nc.dram_tensor("w1d", [128, 9 * C], BF, kind="ExternalInput")
    w2_d = nc.dram_tensor("w2d", [128, 9 * C], BF, kind="ExternalInput")
    w3_d = nc.dram_tensor("w3d", [128, 9 * C], BF, kind="ExternalInput")
    b1_d = nc.dram_tensor("b1d", [128, 1], F32, kind="ExternalInput")
    b2_d = nc.dram_tensor("b2d", [128, 9], F32, kind="ExternalInput")
    b3_d = nc.dram_tensor("b3d", [128, 1], F32, kind="ExternalInput")
    out_d = nc.dram_tensor("out", [C, 2 * HR, W], F32, kind="ExternalOutput")

    with TileContext(nc) as tc:
        with (
            tc.tile_pool(name="big", bufs=1) as big,
            tc.tile_pool(name="wpool", bufs=1) as wpool,
            tc.tile_pool(name="kev", bufs=6) as kevp,
            tc.tile_pool(name="prod", bufs=6) as prodp,
            tc.tile_pool(name="accs", bufs=4) as accp,
            tc.tile_pool(name="stag", bufs=3) as stagp,
            tc.tile_pool(name="ps13", bufs=2, space="PSUM") as ps13p,
            tc.tile_pool(name="psC", bufs=2, space="PSUM") as psCp,
        ):
            ypad = big.tile([128, PR, PW], BF)
            xpad = big.tile([128, PR, PW], BF)
            xswp = big.tile([128, PR, PW], BF)
            a2 = big.tile([128, NWIN * WPX], BF)
            rpn = big.tile([128, PR, PW], BF)
            rpi = big.tile([128, PR, PW], BF)

            w1t = wpool.tile([128, 9 * C], BF)
            w2t = wpool.tile([128, 9 * C], BF)
            w3t = wpool.tile([128, 9 * C], BF)
            b1t = wpool.tile([128, 1], F32)
            b2t = wpool.tile([128, 9], F32)
            b3t = wpool.tile([128, 1], F32)

            def ldchunk(dst, src, k, q):
                r0, r1 = CHUNKS[k]
                q.dma_start(out=dst[:, r0:r1, :], in_=src[:, r0:r1, :])

            ident = mybir.ActivationFunctionType.Identity

            def conv9(src, wt, psLR, w0):
                """3x3 conv quadrants for adjacent window-pair (w0, w0+1).
                psLR [128,1024]: [:,0:512] win w0 normal; [:,512:1024] win
                w0+1 inverted."""
                r0, r1 = RW * w0, RW * (w0 + 1)
                for t in range(9):
                    dy, dx = _tap_off(t)
                    st, sp = (t == 0), (t == 8)
                    lhsL = wt[0:64, t * C:(t + 1) * C]
                    lhsH = wt[64:128, t * C:(t + 1) * C]
                    nc.tensor.matmul(psLR[0:64, 0:512], lhsL,
                                     src[0:64, r0 + dy:r0 + dy + 4, dx:dx + 128],
                                     start=st, stop=sp, tile_position=(0, 0),
                                     skip_group_check=True)
                    nc.tensor.matmul(psLR[64:128, 0:512], lhsH,
                                     src[64:128, r0 + dy:r0 + dy + 4, dx:dx + 128],
                                     start=st, stop=sp, tile_position=(64, 64),
                                     skip_group_check=True)
                    nc.tensor.matmul(psLR[64:128, 512:1024], lhsL,
                                     src[0:64, r1 + dy:r1 + dy + 4, dx:dx + 128],
                                     start=st, stop=sp, tile_position=(0, 64),
                                     skip_group_check=True)
                    nc.tensor.matmul(psLR[0:64, 512:1024], lhsH,
                                     src[64:128, r1 + dy:r1 + dy + 4, dx:dx + 128],
                                     start=st, stop=sp, tile_position=(64, 0),
                                     skip_group_check=True)

            def conv9_swapped(src, wt, psLR, w0):
                """conv9 but every output lands in the opposite partition
                half — consumes the inverted accumulator without a swap."""
                r0, r1 = RW * w0, RW * (w0 + 1)
                for t in range(9):
                    dy, dx = _tap_off(t)
                    st, sp = (t == 0), (t == 8)
                    lhsL = wt[0:64, t * C:(t + 1) * C]
                    lhsH = wt[64:128, t * C:(t + 1) * C]
                    nc.tensor.matmul(psLR[64:128, 0:512], lhsL,
                                     src[0:64, r0 + dy:r0 + dy + 4, dx:dx + 128],
                                     start=st, stop=sp, tile_position=(0, 64),
                                     skip_group_check=True)
                    nc.tensor.matmul(psLR[0:64, 0:512], lhsH,
                                     src[64:128, r0 + dy:r0 + dy + 4, dx:dx + 128],
                                     start=st, stop=sp, tile_position=(64, 0),
                                     skip_group_check=True)
                    nc.tensor.matmul(psLR[0:64, 512:1024], lhsL,
                                     src[0:64, r1 + dy:r1 + dy + 4, dx:dx + 128],
                                     start=st, stop=sp, tile_position=(0, 0),
                                     skip_group_check=True)
                    nc.tensor.matmul(psLR[64:128, 512:1024], lhsH,
                                     src[64:128, r1 + dy:r1 + dy + 4, dx:dx + 128],
                                     start=st, stop=sp, tile_position=(64, 64),
                                     skip_group_check=True)

            def conv1_pair(p):
                w0 = 2 * p
                psLR = ps13p.tile([128, 2 * WPX], F32, tag="ps13",
                                  name=f"c1ps{p}")
                conv9(ypad, w1t, psLR, w0)
                nc.scalar.activation(a2[:, w0 * WPX:(w0 + 2) * WPX],
                                     psLR[:, :], ident, bias=b1t[:, 0:1])

            def x2win(xsrc, wA, dy, dx):
                """[128, 2, 4, 128] view covering same-parity windows wA,
                wA+2 at tap offset (dy, dx)."""
                r0 = RW * wA + dy
                return xsrc[:, r0:r0 + 12, dx:dx + 128].rearrange(
                    "p (b r) c -> p b r c", b=3)[:, 0::2]

            def stage_c(wA, cbs=None):
                """conv2 + dynamic multiply-sum for windows (wA, wA+2).
                Parity-split chains; last add of each chain lands in
                rpn/rpi. cbs = {jp: callback} sprinkles conv3 emission
                between jp groups so its PE blocks hide under ACT's lag."""
                wB = wA + 2
                invA = (wA % 2 == 1)
                size = {invA: 5, (not invA): 4}
                cnt = {False: 0, True: 0}
                acc = {False: None, True: None}
                r0 = RW * wA
                rout = {
                    False: rpn[:, r0 + 1:r0 + 13, 1:129].rearrange(
                        "p (b r) c -> p b r c", b=3)[:, 0::2],
                    True: rpi[:, r0 + 1:r0 + 13, 1:129].rearrange(
                        "p (b r) c -> p b r c", b=3)[:, 0::2],
                }

                for jp in range(5):
                    ta = 2 * jp
                    taps = (ta,) if ta == 8 else (ta, ta + 1)
                    psA = psCp.tile([128, 2 * WPX], F32, tag="ps2",
                                    name=f"psA{wA}_{jp}")
                    lhsA_l = w2t[0:64, ta * C:(ta + 1) * C]
                    lhsA_h = w2t[64:128, ta * C:(ta + 1) * C]
                    psB = None
                    if len(taps) == 2:
                        tb = ta + 1
                        psB = psCp.tile([128, 2 * WPX], F32, tag="ps2",
                                        name=f"psB{wA}_{jp}")
                        lhsB_l = w2t[0:64, tb * C:(tb + 1) * C]
                        lhsB_h = w2t[64:128, tb * C:(tb + 1) * C]
                    for wi, wv in ((0, wA), (1, wB)):
                        awin = a2[:, wv * WPX:(wv + 1) * WPX]
                        sl = slice(wi * WPX, (wi + 1) * WPX)
                        nc.tensor.matmul(psA[0:64, sl], lhsA_l, awin[0:64, :],
                                         tile_position=(0, 0),
                                         skip_group_check=True)
                        nc.tensor.matmul(psA[64:128, sl], lhsA_h,
                                         awin[64:128, :],
                                         tile_position=(64, 64),
                                         skip_group_check=True)
                        if psB is not None:
                            nc.tensor.matmul(psB[64:128, sl], lhsB_l,
                                             awin[0:64, :],
                                             tile_position=(0, 64),
                                             skip_group_check=True)
                            nc.tensor.matmul(psB[0:64, sl], lhsB_h,
                                             awin[64:128, :],
                                             tile_position=(64, 0),
                                             skip_group_check=True)

                    for t in taps:
                        ps = psA if t == ta else psB
                        inv = invA if t == ta else (not invA)
                        dy, dx = _tap_off(t)
                        xop = x2win(xswp if inv else xpad, wA, dy, dx)
                        cnt[inv] += 1
                        first = (cnt[inv] == 1)
                        last = (cnt[inv] == size[inv])
                        if first:
                            acc[inv] = accp.tile([128, 2 * WPX], BF, tag="acc",
                                                 name=f"ac{wA}_{int(inv)}")
                            tgt = acc[inv]
                        else:
                            tgt = prodp.tile([128, 2 * WPX], BF, tag="prod",
                                             name=f"pr{wA}_{t}")
                        tgt4 = tgt[:, :].rearrange("p (b r f) -> p b r f",
                                                   b=2, r=4)
                        kev = kevp.tile([128, 2 * WPX], BF, tag="kev",
                                        name=f"kv{wA}_{t}")
                        nc.scalar.activation(kev[:, :], ps[:, :], ident,
                                             bias=b2t[:, t:t + 1])
                        nc.vector.tensor_mul(
                            out=tgt4,
                            in0=kev[:, :].rearrange(
                                "p (b r f) -> p b r f", b=2, r=4),
                            in1=xop)
                        if not first:
                            if last:
                                nc.vector.tensor_add(
                                    out=rout[inv],
                                    in0=acc[inv][:, :].rearrange(
                                        "p (b r f) -> p b r f", b=2, r=4),
                                    in1=tgt4)
                            else:
                                nc.vector.tensor_add(out=acc[inv][:, :],
                                                     in0=acc[inv][:, :],
                                                     in1=tgt[:, :])
                    if cbs and jp in cbs:
                        cbs[jp]()

            def halosA():
                # available after stage_c(0): rpn win0 h1 row0, rpi h1 row0
                nc.sync.dma_start(out=rpn[0:64, 65:66, :],
                                  in_=rpn[64:128, 1:2, :])
                nc.sync.dma_start(out=rpi[64:128, 65:66, :],
                                  in_=rpi[0:64, 1:2, :])

            def halosB():
                # available after stage_c(13): win15 h0 row63
                nc.sync.dma_start(out=rpn[64:128, 0:1, :],
                                  in_=rpn[0:64, 64:65, :])
                nc.sync.dma_start(out=rpi[0:64, 0:1, :],
                                  in_=rpi[64:128, 64:65, :])

            # conv3 over rpn + rpi for windows (2p, 2p+1); the two source
            # streams accumulate in separate PSUM tiles, combined at
            # eviction: st = (psN + b3) [ACT], st += psI [DVE]. Split into
            # three emission pieces so the MM blocks can hide inside
            # stage_c's jp cadence.
            c3ps = {}

            def c3n(p, pool=ps13p, tg="ps13"):
                psN = pool.tile([128, 2 * WPX], F32, tag=tg, name=f"c3n{p}")
                conv9(rpn, w3t, psN, 2 * p)
                c3ps[p] = psN

            def c3i(p, pool=ps13p, tg="ps13"):
                psI = pool.tile([128, 2 * WPX], F32, tag=tg, name=f"c3i{p}")
                conv9_swapped(rpi, w3t, psI, 2 * p)
                c3ps[p] = (c3ps[p], psI)

            def c3fin(p):
                psN, psI = c3ps.pop(p)
                w0 = 2 * p
                st_t = stagp.tile([128, 2 * WPX], F32, tag="stag",
                                  name=f"st{p}")
                nc.scalar.activation(st_t[:, :], psN[:, :], ident,
                                     bias=b3t[:, 0:1])
                nc.vector.tensor_add(out=st_t[:, :], in0=st_t[:, :],
                                     in1=psI[:, :])
                ra, rb = RW * w0, RW * (w0 + 1)
                nc.sync.dma_start(out=out_d[:, ra:ra + 4, :],
                                  in_=st_t[0:64, 0:512])
                nc.sync.dma_start(out=out_d[:, HR + ra:HR + ra + 4, :],
                                  in_=st_t[64:128, 0:512])
                nc.sync.dma_start(out=out_d[:, HR + rb:HR + rb + 4, :],
                                  in_=st_t[0:64, 512:1024])
                nc.sync.dma_start(out=out_d[:, rb:rb + 4, :],
                                  in_=st_t[64:128, 512:1024])

            def conv3_pair(p, pool=ps13p, tg="ps13"):
                c3n(p, pool, tg)
                c3i(p, pool, tg)
                c3fin(p)

            # ---- emission schedule ----
            ldchunk(ypad, yp_d, 0, nc.sync)
            for td, tt in ((w1_d, w1t), (w2_d, w2t), (w3_d, w3t),
                           (b1_d, b1t), (b2_d, b2t), (b3_d, b3t)):
                nc.sync.dma_start(out=tt[:], in_=td[:])

            nc.vector.memset(rpn[0:64, 0:1, :], 0.0)
            nc.vector.memset(rpn[64:128, 65:66, :], 0.0)
            nc.vector.memset(rpn[:, :, 0:1], 0.0)
            nc.vector.memset(rpn[:, :, 129:130], 0.0)
            nc.vector.memset(rpi[0:64, 65:66, :], 0.0)
            nc.vector.memset(rpi[64:128, 0:1, :], 0.0)
            nc.vector.memset(rpi[:, :, 0:1], 0.0)
            nc.vector.memset(rpi[:, :, 129:130], 0.0)

            for k in range(4):
                ldchunk(xpad, xp_d, k, nc.scalar)
                ldchunk(xswp, xs_d, k, nc.scalar)
                if k > 0:
                    ldchunk(ypad, yp_d, k, nc.sync)

            conv1_pair(0)
            conv1_pair(1)
            conv1_pair(2)
            conv1_pair(3)
            stage_c(0)
            halosA()
            conv1_pair(4)
            stage_c(1)
            conv1_pair(5)
            stage_c(4)
            conv1_pair(6)
            stage_c(5)
            conv1_pair(7)
            stage_c(8)
            conv3_pair(1)
            stage_c(9)
            conv3_pair(2)
            stage_c(13)
            halosB()
            conv3_pair(0)
            conv3_pair(3)
            stage_c(12)
            conv3_pair(4)
            conv3_pair(5, psCp, "ps2")
            conv3_pair(6)
            conv3_pair(7, psCp, "ps2")

    nc.compile()
    return nc


def _prep_weights(w_gk1, b_gk1, w_gk2, b_gk2, w_fuse, b_fuse):
    bf = ml_dtypes.bfloat16

    def conv_lhst(wc):
        l = np.empty((128, 9 * C), dtype=bf)
        for t in range(9):
            dy, dx = _tap_off(t)
            m = wc[:, :, dy, dx].T.astype(bf)  # [I, O] lhsT
            l[0:64, t * C:(t + 1) * C] = m
            l[64:128, t * C:(t + 1) * C] = m
        return l

    w1d = conv_lhst(np.asarray(w_gk1))
    w3d = conv_lhst(np.asarray(w_fuse))

    w2 = np.asarray(w_gk2).reshape(C * 9, C)
    w2d = np.empty((128, 9 * C), dtype=bf)
    for t in range(9):
        m = w2[t::9, :].T.astype(bf)
        w2d[0:64, t * C:(t + 1) * C] = m
        w2d[64:128, t * C:(t + 1) * C] = m

    b1 = np.asarray(b_gk1, np.float32)
    b3 = np.asarray(b_fuse, np.float32)
    b1d = np.concatenate([b1, b1]).reshape(128, 1)
    b3d = np.concatenate([b3, b3]).reshape(128, 1)
    b2 = np.asarray(b_gk2, np.float32).reshape(C, 9)
    b2d = np.concatenate([b2, b2], axis=0)
    return w1d, w2d, w3d, b1d, b2d, b3d


def _prep_pad(img):
    """[64,128,128] fp32 -> ([128,66,130] bf16 padded, halves-swapped)."""
    bf = ml_dtypes.bfloat16
    b = np.asarray(img, np.float32).astype(bf)
    pad = np.zeros((128, PR, PW), dtype=bf)
    pad[0:64, 1:66, 1:129] = b[:, 0:65]       # h0: img rows 0..64
    pad[64:128, 0:65, 1:129] = b[:, 63:128]   # h1: img rows 63..127
    swp = np.ascontiguousarray(
        np.concatenate([pad[64:128], pad[0:64]], axis=0))
    return pad, swp


def kernel(x, y, w_gk1, b_gk1, w_gk2, b_gk2, w_fuse, b_fuse):
    from concourse.bass_utils import run_bass_kernel_spmd

    with _lock:
        if "nc" not in _cache:
            _cache["nc"] = build_nc()
    nc = _cache["nc"]

    w1d, w2d, w3d, b1d, b2d, b3d = _prep_weights(
        w_gk1, b_gk1, w_gk2, b_gk2, w_fuse, b_fuse)

    x = np.asarray(x, np.float32)
    y = np.asarray(y, np.float32)
    n = x.shape[0]
    assert n == 8, f"expected batch 8, got {n}"
    in_maps = []
    for i in range(n):
        xpd, xsd = _prep_pad(x[i])
        ypd, _ = _prep_pad(y[i])
        in_maps.append({
            "xpd": xpd, "xsd": xsd, "ypd": ypd,
            "w1d": w1d, "w2d": w2d, "w3d": w3d,
            "b1d": b1d, "b2d": b2d, "b3d": b3d,
        })
    res = run_bass_kernel_spmd(nc, in_maps, core_ids=list(range(n)))
    return np.stack([res.results[i]["out"] for i in range(n)], axis=0)


# revision 47
# speedup vs baseline: 1.1864x; 1.0024x over previous
"""DepthDC3x3 dynamic depthwise conv — Trainium2 Bass kernel, 8 NeuronCores.

Per-sample pipeline (data-parallel over batch N=8 -> one sample per core):
  A  = conv3x3(y, w_gk1) + b_gk1                    # [64,128,128]
  K  = conv1x1(A, w_gk2) + b_gk2                    # [576,128,128] -> per-tap
  r  = sum_k K[:,k] * shift_k(x)                    # dynamic depthwise 3x3
  out= conv3x3(r, w_fuse) + b_fuse                  # [64,128,128]

Key design points (v6):
  * Half-split layout: SBUF partitions = (channel 0..63) x (image half).
  * Inputs are pre-padded + pre-cast to bf16 on the HOST into the exact
    on-chip layout ([128 part, 66, 130], plus a halves-swapped copy of x),
    so the device does zero casts/swaps: 12 chunked HWDGE loads (y on the
    sync ring, x + xswp on the scalar ring) feed compute directly.
  * conv1 window-pairs produce one normal + one halves-swapped ("inverted")
    PSUM tile; inversion is never fixed up downstream.
  * Stage C: conv2 PSUM tiles are [128,1024] 2-bank (windows w, w+2, same
    parity) -> ONE ACT eviction and ONE DVE multiply per tap. Separate
    accumulation chains per parity; each chain's last add writes straight
    into the padded accumulator (rpn / rpi) - no fold, no swap DMA.
  * conv3 exploits linearity: 9 taps over rpn + 9 over rpi (own PSUM tile,
    quadrant-swapped outputs), combined at eviction (ACT evict + DVE add).
    conv3 pairs 1-4 run interleaved with late stage_c's; only pair 0 needs
    the last halo, so the tail is short.
  * bf16 matmuls, 2x2 tile_position quadrants -> 4x PE concurrency.
"""

import threading

import numpy as np
import ml_dtypes

import concourse.bacc as bacc
import concourse.mybir as mybir
from concourse.tile import TileContext

BF = mybir.dt.bfloat16
F32 = mybir.dt.float32

C = 64            # channels
HR = 64           # rows per half
W = 128           # image width
PW = W + 2        # padded width
PR = HR + 2       # padded rows per half
NWIN = 16         # windows per half (4 rows x 128 cols each)
WPX = 512
RW = 4            # rows per window
KS = 3

# row chunks of the padded-row index space (0..66)
CHUNKS = ((0, 18), (18, 34), (34, 50), (50, 66))

_lock = threading.Lock()
_cache = {}


def _tap_off(t):
    return divmod(t, KS)


def build_nc():
    nc = bacc.Bacc("TRN2", target_bir_lowering=False, debug=False)

    xp_d = nc.dram_tensor("xpd", [128, PR, PW], BF, kind="ExternalInput")
    xs_d = nc.dram_tensor("xsd", [128, PR, PW], BF, kind="ExternalInput")
    yp_d = nc.dram_tensor("ypd", [128, PR, PW], BF, kind="ExternalInput")
    wall_d = nc.dram_tensor("wall", [128, 27 * C], BF, kind="ExternalInput")
    ball_d = nc.dram_tensor("ball", [128, 11], F32, kind="ExternalInput")
    out_d = nc.dram_tensor("out", [C, 2 * HR, W], F32, kind="ExternalOutput")

    with TileContext(nc) as tc:
        with (
            tc.tile_pool(name="big", bufs=1) as big,
            tc.tile_pool(name="wpool", bufs=1) as wpool,
            tc.tile_pool(name="kev", bufs=6) as kevp,
            tc.tile_pool(name="prod", bufs=6) as prodp,
            tc.tile_pool(name="accs", bufs=4) as accp,
            tc.tile_pool(name="stag", bufs=3) as stagp,
            tc.tile_pool(name="ps13", bufs=2, space="PSUM") as ps13p,
            tc.tile_pool(name="psC", bufs=2, space="PSUM") as psCp,
        ):
            ypad = big.tile([128, PR, PW], BF)
            xpad = big.tile([128, PR, PW], BF)
            xswp = big.tile([128, PR, PW], BF)
            a2 = big.tile([128, NWIN * WPX], BF)
            rpn = big.tile([128, PR, PW], BF)
            rpi = big.tile([128, PR, PW], BF)

            wall = wpool.tile([128, 27 * C], BF)
            ball = wpool.tile([128, 11], F32)
            W2O, W3O = 9 * C, 18 * C

            def ldchunk(dst, src, k, q):
                r0, r1 = CHUNKS[k]
                q.dma_start(out=dst[:, r0:r1, :], in_=src[:, r0:r1, :])

            ident = mybir.ActivationFunctionType.Identity

            def conv9(src, woff, psLR, w0):
                """3x3 conv quadrants for adjacent window-pair (w0, w0+1).
                psLR [128,1024]: [:,0:512] win w0 normal; [:,512:1024] win
                w0+1 inverted."""
                r0, r1 = RW * w0, RW * (w0 + 1)
                for t in range(9):
                    dy, dx = _tap_off(t)
                    st, sp = (t == 0), (t == 8)
                    lhsL = wall[0:64, woff + t * C:woff + (t + 1) * C]
                    lhsH = wall[64:128, woff + t * C:woff + (t + 1) * C]
                    nc.tensor.matmul(psLR[0:64, 0:512], lhsL,
                                     src[0:64, r0 + dy:r0 + dy + 4, dx:dx + 128],
                                     start=st, stop=sp, tile_position=(0, 0),
                                     skip_group_check=True)
                    nc.tensor.matmul(psLR[64:128, 0:512], lhsH,
                                     src[64:128, r0 + dy:r0 + dy + 4, dx:dx + 128],
                                     start=st, stop=sp, tile_position=(64, 64),
                                     skip_group_check=True)
                    nc.tensor.matmul(psLR[64:128, 512:1024], lhsL,
                                     src[0:64, r1 + dy:r1 + dy + 4, dx:dx + 128],
                                     start=st, stop=sp, tile_position=(0, 64),
                                     skip_group_check=True)
                    nc.tensor.matmul(psLR[0:64, 512:1024], lhsH,
                                     src[64:128, r1 + dy:r1 + dy + 4, dx:dx + 128],
                                     start=st, stop=sp, tile_position=(64, 0),
                                     skip_group_check=True)

            def conv9_swapped(src, woff, psLR, w0):
                """conv9 but every output lands in the opposite partition
                half — consumes the inverted accumulator without a swap."""
                r0, r1 = RW * w0, RW * (w0 + 1)
                for t in range(9):
                    dy, dx = _tap_off(t)
                    st, sp = (t == 0), (t == 8)
                    lhsL = wall[0:64, woff + t * C:woff + (t + 1) * C]
                    lhsH = wall[64:128, woff + t * C:woff + (t + 1) * C]
                    nc.tensor.matmul(psLR[64:128, 0:512], lhsL,
                                     src[0:64, r0 + dy:r0 + dy + 4, dx:dx + 128],
                                     start=st, stop=sp, tile_position=(0, 64),
                                     skip_group_check=True)
                    nc.tensor.matmul(psLR[0:64, 0:512], lhsH,
                                     src[64:128, r0 + dy:r0 + dy + 4, dx:dx + 128],
                                     start=st, stop=sp, tile_position=(64, 0),
                                     skip_group_check=True)
                    nc.tensor.matmul(psLR[0:64, 512:1024], lhsL,
                                     src[0:64, r1 + dy:r1 + dy + 4, dx:dx + 128],
                                     start=st, stop=sp, tile_position=(0, 0),
                                     skip_group_check=True)
                    nc.tensor.matmul(psLR[64:128, 512:1024], lhsH,
                                     src[64:128, r1 + dy:r1 + dy + 4, dx:dx + 128],
                                     start=st, stop=sp, tile_position=(64, 64),
                                     skip_group_check=True)

            def conv1_pair(p):
                w0 = 2 * p
                psLR = ps13p.tile([128, 2 * WPX], F32, tag="ps13",
                                  name=f"c1ps{p}")
                conv9(ypad, 0, psLR, w0)
                nc.scalar.activation(a2[:, w0 * WPX:(w0 + 2) * WPX],
                                     psLR[:, :], ident, bias=ball[:, 0:1])

            def x2win(xsrc, wA, dy, dx):
                """[128, 2, 4, 128] view covering same-parity windows wA,
                wA+2 at tap offset (dy, dx)."""
                r0 = RW * wA + dy
                return xsrc[:, r0:r0 + 12, dx:dx + 128].rearrange(
                    "p (b r) c -> p b r c", b=3)[:, 0::2]

            def stage_c(wA, cbs=None):
                """conv2 + dynamic multiply-sum for windows (wA, wA+2).
                Parity-split chains; last add of each chain lands in
                rpn/rpi. cbs = {jp: callback} sprinkles conv3 emission
                between jp groups so its PE blocks hide under ACT's lag."""
                wB = wA + 2
                invA = (wA % 2 == 1)
                size = {invA: 5, (not invA): 4}
                cnt = {False: 0, True: 0}
                acc = {False: None, True: None}
                r0 = RW * wA
                rout = {
                    False: rpn[:, r0 + 1:r0 + 13, 1:129].rearrange(
                        "p (b r) c -> p b r c", b=3)[:, 0::2],
                    True: rpi[:, r0 + 1:r0 + 13, 1:129].rearrange(
                        "p (b r) c -> p b r c", b=3)[:, 0::2],
                }

                for jp in range(5):
                    ta = 2 * jp
                    taps = (ta,) if ta == 8 else (ta, ta + 1)
                    psA = psCp.tile([128, 2 * WPX], F32, tag="ps2",
                                    name=f"psA{wA}_{jp}")
                    lhsA_l = wall[0:64, W2O + ta * C:W2O + (ta + 1) * C]
                    lhsA_h = wall[64:128, W2O + ta * C:W2O + (ta + 1) * C]
                    psB = None
                    if len(taps) == 2:
                        tb = ta + 1
                        psB = psCp.tile([128, 2 * WPX], F32, tag="ps2",
                                        name=f"psB{wA}_{jp}")
                        lhsB_l = wall[0:64, W2O + tb * C:W2O + (tb + 1) * C]
                        lhsB_h = wall[64:128, W2O + tb * C:W2O + (tb + 1) * C]
                    for wi, wv in ((0, wA), (1, wB)):
                        awin = a2[:, wv * WPX:(wv + 1) * WPX]
                        sl = slice(wi * WPX, (wi + 1) * WPX)
                        nc.tensor.matmul(psA[0:64, sl], lhsA_l, awin[0:64, :],
                                         tile_position=(0, 0),
                                         skip_group_check=True)
                        nc.tensor.matmul(psA[64:128, sl], lhsA_h,
                                         awin[64:128, :],
                                         tile_position=(64, 64),
                                         skip_group_check=True)
                        if psB is not None:
                            nc.tensor.matmul(psB[64:128, sl], lhsB_l,
                                             awin[0:64, :],
                                             tile_position=(0, 64),
                                             skip_group_check=True)
                            nc.tensor.matmul(psB[0:64, sl], lhsB_h,
                                             awin[64:128, :],
                                             tile_position=(64, 0),
                                             skip_group_check=True)

                    for t in taps:
                        ps = psA if t == ta else psB
                        inv = invA if t == ta else (not invA)
                        dy, dx = _tap_off(t)
                        xop = x2win(xswp if inv else xpad, wA, dy, dx)
                        cnt[inv] += 1
                        first = (cnt[inv] == 1)
                        last = (cnt[inv] == size[inv])
                        if first:
                            acc[inv] = accp.tile([128, 2 * WPX], BF, tag="acc",
                                                 name=f"ac{wA}_{int(inv)}")
                            tgt = acc[inv]
                        else:
                            tgt = prodp.tile([128, 2 * WPX], BF, tag="prod",
                                             name=f"pr{wA}_{t}")
                        tgt4 = tgt[:, :].rearrange("p (b r f) -> p b r f",
                                                   b=2, r=4)
                        kev = kevp.tile([128, 2 * WPX], BF, tag="kev",
                                        name=f"kv{wA}_{t}")
                        nc.scalar.activation(kev[:, :], ps[:, :], ident,
                                             bias=ball[:, 1 + t:2 + t])
                        nc.vector.tensor_mul(
                            out=tgt4,
                            in0=kev[:, :].rearrange(
                                "p (b r f) -> p b r f", b=2, r=4),
                            in1=xop)
                        if not first:
                            if last:
                                nc.vector.tensor_add(
                                    out=rout[inv],
                                    in0=acc[inv][:, :].rearrange(
                                        "p (b r f) -> p b r f", b=2, r=4),
                                    in1=tgt4)
                            else:
                                nc.vector.tensor_add(out=acc[inv][:, :],
                                                     in0=acc[inv][:, :],
                                                     in1=tgt[:, :])
                    if cbs and jp in cbs:
                        cbs[jp]()

            def halosA():
                # available after stage_c(0): rpn win0 h1 row0, rpi h1 row0
                nc.sync.dma_start(out=rpn[0:64, 65:66, :],
                                  in_=rpn[64:128, 1:2, :])
                nc.sync.dma_start(out=rpi[64:128, 65:66, :],
                                  in_=rpi[0:64, 1:2, :])

            def halosB():
                # available after stage_c(13): win15 h0 row63
                nc.sync.dma_start(out=rpn[64:128, 0:1, :],
                                  in_=rpn[0:64, 64:65, :])
                nc.sync.dma_start(out=rpi[0:64, 0:1, :],
                                  in_=rpi[64:128, 64:65, :])

            # conv3 over rpn + rpi for windows (2p, 2p+1); the two source
            # streams accumulate in separate PSUM tiles, combined at
            # eviction: st = (psN + b3) [ACT], st += psI [DVE]. Split into
            # three emission pieces so the MM blocks can hide inside
            # stage_c's jp cadence.
            c3ps = {}

            def c3n(p, pool=ps13p, tg="ps13"):
                psN = pool.tile([128, 2 * WPX], F32, tag=tg, name=f"c3n{p}")
                conv9(rpn, W3O, psN, 2 * p)
                c3ps[p] = psN

            def c3i(p, pool=ps13p, tg="ps13"):
                psI = pool.tile([128, 2 * WPX], F32, tag=tg, name=f"c3i{p}")
                conv9_swapped(rpi, W3O, psI, 2 * p)
                c3ps[p] = (c3ps[p], psI)

            def c3fin(p):
                psN, psI = c3ps.pop(p)
                w0 = 2 * p
                st_t = stagp.tile([128, 2 * WPX], F32, tag="stag",
                                  name=f"st{p}")
                nc.scalar.activation(st_t[:, :], psN[:, :], ident,
                                     bias=ball[:, 10:11])
                nc.vector.tensor_add(out=st_t[:, :], in0=st_t[:, :],
                                     in1=psI[:, :])
                ra, rb = RW * w0, RW * (w0 + 1)
                nc.sync.dma_start(out=out_d[:, ra:ra + 4, :],
                                  in_=st_t[0:64, 0:512])
                nc.sync.dma_start(out=out_d[:, HR + ra:HR + ra + 4, :],
                                  in_=st_t[64:128, 0:512])
                nc.sync.dma_start(out=out_d[:, HR + rb:HR + rb + 4, :],
                                  in_=st_t[0:64, 512:1024])
                nc.sync.dma_start(out=out_d[:, rb:rb + 4, :],
                                  in_=st_t[64:128, 512:1024])

            def conv3_pair(p, pool=ps13p, tg="ps13"):
                c3n(p, pool, tg)
                c3i(p, pool, tg)
                c3fin(p)

            # ---- emission schedule ----
            nc.sync.dma_start(out=ypad[:, 0:10, :], in_=yp_d[:, 0:10, :])
            nc.sync.dma_start(out=wall[:], in_=wall_d[:])
            nc.sync.dma_start(out=ball[:], in_=ball_d[:])
            nc.sync.dma_start(out=ypad[:, 10:18, :], in_=yp_d[:, 10:18, :])

            nc.vector.memset(rpn[0:64, 0:1, :], 0.0)
            nc.vector.memset(rpn[64:128, 65:66, :], 0.0)
            nc.vector.memset(rpn[:, :, 0:1], 0.0)
            nc.vector.memset(rpn[:, :, 129:130], 0.0)
            nc.vector.memset(rpi[0:64, 65:66, :], 0.0)
            nc.vector.memset(rpi[64:128, 0:1, :], 0.0)
            nc.vector.memset(rpi[:, :, 0:1], 0.0)
            nc.vector.memset(rpi[:, :, 129:130], 0.0)

            for k in range(4):
                ldchunk(xpad, xp_d, k, nc.scalar)
                ldchunk(xswp, xs_d, k, nc.scalar)
                if k > 0:
                    ldchunk(ypad, yp_d, k, nc.sync)

            conv1_pair(0)
            conv1_pair(1)
            conv1_pair(2)
            conv1_pair(3)
            stage_c(0)
            halosA()
            conv1_pair(4)
            stage_c(1)
            conv1_pair(5)
            stage_c(4)
            conv1_pair(6)
            stage_c(5)
            conv1_pair(7)
            stage_c(8)
            conv3_pair(1)
            stage_c(9)
            conv3_pair(2)
            stage_c(13)
            halosB()
            conv3_pair(0)
            conv3_pair(3)
            stage_c(12)
            conv3_pair(4)
            conv3_pair(5, psCp, "ps2")
            conv3_pair(6)
            conv3_pair(7, psCp, "ps2")

    nc.compile()
    return nc


def _prep_weights(w_gk1, b_gk1, w_gk2, b_gk2, w_fuse, b_fuse):
    bf = ml_dtypes.bfloat16

    def conv_lhst(wc):
        l = np.empty((128, 9 * C), dtype=bf)
        for t in range(9):
            dy, dx = _tap_off(t)
            m = wc[:, :, dy, dx].T.astype(bf)  # [I, O] lhsT
            l[0:64, t * C:(t + 1) * C] = m
            l[64:128, t * C:(t + 1) * C] = m
        return l

    w1d = conv_lhst(np.asarray(w_gk1))
    w3d = conv_lhst(np.asarray(w_fuse))

    w2 = np.asarray(w_gk2).reshape(C * 9, C)
    w2d = np.empty((128, 9 * C), dtype=bf)
    for t in range(9):
        m = w2[t::9, :].T.astype(bf)
        w2d[0:64, t * C:(t + 1) * C] = m
        w2d[64:128, t * C:(t + 1) * C] = m

    b1 = np.asarray(b_gk1, np.float32)
    b3 = np.asarray(b_fuse, np.float32)
    b1d = np.concatenate([b1, b1]).reshape(128, 1)
    b3d = np.concatenate([b3, b3]).reshape(128, 1)
    b2 = np.asarray(b_gk2, np.float32).reshape(C, 9)
    b2d = np.concatenate([b2, b2], axis=0)
    wall = np.ascontiguousarray(np.concatenate([w1d, w2d, w3d], axis=1))
    ball = np.ascontiguousarray(
        np.concatenate([b1d, b2d, b3d], axis=1).astype(np.float32))
    return wall, ball


def _prep_pad(img):
    """[64,128,128] fp32 -> ([128,66,130] bf16 padded, halves-swapped)."""
    bf = ml_dtypes.bfloat16
    b = np.asarray(img, np.float32).astype(bf)
    pad = np.zeros((128, PR, PW), dtype=bf)
    pad[0:64, 1:66, 1:129] = b[:, 0:65]       # h0: img rows 0..64
    pad[64:128, 0:65, 1:129] = b[:, 63:128]   # h1: img rows 63..127
    swp = np.ascontiguousarray(
        np.concatenate([pad[64:128], pad[0:64]], axis=0))
    return pad, swp


def kernel(x, y, w_gk1, b_gk1, w_gk2, b_gk2, w_fuse, b_fuse):
    from concourse.bass_utils import run_bass_kernel_spmd

    with _lock:
        if "nc" not in _cache:
            _cache["nc"] = build_nc()
    nc = _cache["nc"]

    wall, ball = _prep_weights(
        w_gk1, b_gk1, w_gk2, b_gk2, w_fuse, b_fuse)

    x = np.asarray(x, np.float32)
    y = np.asarray(y, np.float32)
    n = x.shape[0]
    assert n == 8, f"expected batch 8, got {n}"
    in_maps = []
    for i in range(n):
        xpd, xsd = _prep_pad(x[i])
        ypd, _ = _prep_pad(y[i])
        in_maps.append({
            "xpd": xpd, "xsd": xsd, "ypd": ypd,
            "wall": wall, "ball": ball,
        })
    res = run_bass_kernel_spmd(nc, in_maps, core_ids=list(range(n)))
    return np.stack([res.results[i]["out"] for i in range(n)], axis=0)
